# revision 1
# baseline (speedup 1.0000x reference)
"""Self-contained MixHop Trainium2 kernel: 8-core SPMD Bass program.

kernel(**inputs) takes the FULL inputs (as produced by setup_inputs) and
returns the FULL [100000, 40] float32 output.  Internally: nodes are sharded
across 8 NeuronCores (degree-sorted, class-colored for gather efficiency),
each GCN hop is dma_gather + segmented reduce + per-row scaling, with a
per-hop AllGather of the 3.2MB shard state; dense lin/LN/GELU stay
data-parallel per core.
"""

import sys

import numpy as np

# the merged preprocessing code below was written against `prep.X`
prep = sys.modules[__name__]

"""Host-side preprocessing for the MixHop Trainium kernel.

Design:
- GCN norm is separable: norm_ij = dis[i]*dis[j].  Keep state t_k = dis * s_k.
  Then s_{k+1} = dis * ((A_raw + I) @ t_k)  and  t_{k+1} = dis^2 * ((A_raw+I) @ t_k).
  Each hop is: gather rows of t_k by src, plain segmented sum per dst, scale.
- Nodes are assigned to 8 cores, degree-sorted (desc) and snake-striped for
  balance.  Core k owns 12500 real slots; padded to 12544 = 98*128 slot grid,
  plus one zero row -> SHARD = 12545 rows per core in the all-gathered state.
- Gather uses dma_gather with int16 indices.  To cover 8*12545 = 100360 rows
  with int16, the state is viewed as 4 interleaved "classes" with
  elem_step = 256 floats (1024B): class c covers rows {4*i + c}, idx = i.
  class(node) = (core + slot) % 4, so the within-tile position (mod 4) decides
  a node's class; a greedy coloring balances every dst's in-neighbour classes.
- Edges are bucketed per (dst tile of 128 slots, class).  Within a batch of
  T tiles, each class c is padded to a rectangle [128, T, D_bc] where
  D_bc = max class-c in-degree over the batch ACROSS ALL CORES (SPMD: one
  program, shared shapes); pad entries point at a zero row of matching class.
- Self loops are NOT in the tables; the device adds the dst's own t_k row
  (loaded contiguously from the local shard) after the class reduction.
"""

import numpy as np

NCORES = 8
P = 128
N = 100000
IN_F = 128
HID = 64
OUT_F = 40
POWERS = [6, 8, 10]
MAXP = 10
LN_EPS = 1e-5

TILES = 98                 # per-core dst tiles
SLOTS = TILES * P          # 12544 padded slots per core
REAL = N // NCORES         # 12500 real nodes per core
SHARD = SLOTS + 1          # + 1 zero row = 12545  (SHARD % 4 == 1)
TOT = NCORES * SHARD       # 100360 all-gathered rows
ZROW = SLOTS               # zero row slot within a shard


def set_dims(n, tiles):
    """Reconfigure for a smaller debug instance (keeps NCORES/feature dims)."""
    global N, TILES, SLOTS, REAL, SHARD, TOT, ZROW
    N = n
    TILES = tiles
    SLOTS = TILES * P
    REAL = N // NCORES
    assert REAL * NCORES == N and REAL <= SLOTS
    SHARD = SLOTS + 1
    TOT = NCORES * SHARD
    ZROW = SLOTS


def zero_idx(c):
    """Gather index of a zero row for class c (row SHARD*c + SLOTS)."""
    z = SHARD * c + SLOTS
    assert z % 4 == c, (c, z)
    return z // 4


def _assign_nodes(deg):
    """Degree-sorted snake assignment: node -> (core, tile, degree rank)."""
    order = np.argsort(-deg, kind="stable")
    node_core = np.empty(N, np.int32)
    node_rank = np.empty(N, np.int32)  # rank within core (= provisional slot)
    snake = np.tile(
        np.concatenate([np.arange(NCORES), np.arange(NCORES - 1, -1, -1)]),
        N // (2 * NCORES) + 1,
    )[:N]
    node_core[order] = snake
    node_rank[order] = np.arange(N) // NCORES
    return node_core, node_rank


def _color_nodes(src, dst, node_core, node_tile, out_deg_order):
    """Greedy 4-coloring balancing each dst's in-neighbour colors, subject to
    32-per-color capacity within each (core, tile)."""
    cnt = np.zeros((N, 4), np.int32)
    cap = np.full((NCORES, TILES, 4), 32, np.int32)
    color = np.full(N, -1, np.int8)

    # CSR of out-neighbours by src
    order = np.argsort(src, kind="stable")
    dst_s = dst[order]
    indptr = np.zeros(N + 1, np.int64)
    np.cumsum(np.bincount(src, minlength=N), out=indptr[1:])

    big = np.int32(1 << 20)
    for n in out_deg_order:
        k, t = node_core[n], node_tile[n]
        capn = cap[k, t]
        ds = dst_s[indptr[n] : indptr[n + 1]]
        if len(ds):
            score = cnt[ds].sum(axis=0, dtype=np.int64)
        else:
            score = np.zeros(4, np.int64)
        score = np.where(capn > 0, score, big)
        c = int(np.argmin(score))
        color[n] = c
        capn[c] -= 1
        if len(ds):
            np.add.at(cnt, (ds, c), 1)

    # refinement sweeps: move a node to a color that lowers sum of per-dst maxima
    rng = np.random.default_rng(7)
    for _ in range(2):
        order = rng.permutation(N)
        for n in order:
            ds = dst_s[indptr[n] : indptr[n + 1]]
            if not len(ds):
                continue
            k, t = node_core[n], node_tile[n]
            capn = cap[k, t]
            c_old = color[n]
            sub = cnt[ds]  # [deg, 4]
            mx = sub.max(axis=1)
            # cost delta of removing from c_old: -1 where c_old was unique max
            col_old = sub[:, c_old]
            unique_max = (col_old == mx) & ((sub == mx[:, None]).sum(axis=1) == 1)
            gain_remove = int(unique_max.sum())
            best_c, best_delta = c_old, 0
            for c_new in range(4):
                if c_new == c_old or capn[c_new] <= 0:
                    continue
                add_cost = int((sub[:, c_new] + 1 > mx).sum())
                delta = add_cost - gain_remove
                if delta < best_delta:
                    best_delta, best_c = delta, c_new
            if best_c != c_old:
                np.add.at(cnt, (ds, c_old), -1)
                np.add.at(cnt, (ds, best_c), 1)
                capn[c_old] += 1
                capn[best_c] -= 1
                color[n] = best_c
    return color, cnt


def preprocess(edge_index, t_batch=6, color=True):
    """edge_index: [2, E].  Returns dict with shared geometry + per-core tables."""
    src = np.asarray(edge_index[0], dtype=np.int64)
    dst = np.asarray(edge_index[1], dtype=np.int64)

    deg = np.bincount(dst, minlength=N).astype(np.float64) + 1.0  # + self loop
    dis = (1.0 / np.sqrt(deg)).astype(np.float32)

    node_core, node_rank = _assign_nodes(deg)
    node_tile = node_rank // P

    # --- choose within-tile positions (the mod-4 position fixes the class) ---
    if color:
        out_deg = np.bincount(src, minlength=N)
        odo = np.argsort(-out_deg, kind="stable")
        col, cnt = _color_nodes(src, dst, node_core, node_tile, odo)
    else:
        col = ((node_core + node_rank) % 4).astype(np.int8)
        key0 = dst * 4 + col[src].astype(np.int64)
        cnt = np.bincount(key0, minlength=N * 4).reshape(N, 4).astype(np.int32)

    # Re-tile: per core, per color, sort dsts by their pooled cost (max class
    # in-count) descending and chunk into 32s -> tile = chunk index.  This
    # makes every tile homogeneous in the quantity the rectangles pool over,
    # while the 32-per-color capacity holds by construction.
    max_cnt = cnt.max(axis=1)
    node_slot = np.empty(N, np.int64)
    for k in range(NCORES):
        idx = np.nonzero(node_core == k)[0]
        plist = {c: np.nonzero((k + np.arange(P)) % 4 == c)[0] for c in range(4)}
        for c in range(4):
            nodes_c = idx[col[idx] == c]
            order_c = nodes_c[np.argsort(-max_cnt[nodes_c], kind="stable")]
            t_of = np.arange(len(order_c)) // 32
            r_of = np.arange(len(order_c)) % 32
            assert t_of.max() < TILES
            node_slot[order_c] = t_of * P + plist[c][r_of]

    abs_row = node_core.astype(np.int64) * SHARD + node_slot
    cls_of = (abs_row % 4).astype(np.int64)
    if color:
        assert (cls_of == col.astype(np.int64)).all()

    # --- per-core edge grouping, pass 1: counts for shared geometry ---
    percore = []
    for k in range(NCORES):
        m = node_core[dst] == k
        s_k = src[m]
        d_slot = node_slot[dst[m]]
        e_cls = cls_of[s_k]
        key = d_slot * 4 + e_cls
        sort = np.argsort(key, kind="stable")
        key_s = key[sort]
        idx_s = ((abs_row[s_k] - e_cls) // 4)[sort]
        counts = np.bincount(key_s, minlength=SLOTS * 4).reshape(SLOTS, 4)
        starts = np.concatenate([[0], np.cumsum(counts.reshape(-1))[:-1]])
        rank = np.arange(len(key_s)) - starts[key_s]
        percore.append((key_s, idx_s, rank, counts))

    # per-(tile, class) shared D (max over cores)
    tileD = np.zeros((TILES, 4), np.int64)
    for t in range(TILES):
        lo, hi = t * P, (t + 1) * P
        for c in range(4):
            tileD[t, c] = max(
                max(int(percore[k][3][lo:hi, c].max()) for k in range(NCORES)), 1
            )

    # --- batches: pack tiles while tb * sum_c(max D_c) <= budget cols ---
    budget = t_batch  # interpreted as gather-column budget when > 32
    if budget <= 32:
        budget = 224
    batches = []
    t0 = 0
    while t0 < TILES:
        t1 = t0 + 1
        while t1 < TILES and t1 - t0 < 8:
            Dk = tileD[t0:t1 + 1].max(axis=0)
            if (t1 + 1 - t0) * int(Dk.sum()) > budget:
                break
            t1 += 1
        batches.append((t0, t1))
        t0 = t1

    meta = []
    col_off = 0
    for (t0, t1) in batches:
        tb = t1 - t0
        info = {"t0": t0, "t1": t1, "D": [], "off": [], "n": []}
        for c in range(4):
            D = int(tileD[t0:t1, c].max())
            n = tb * D * P
            info["D"].append(D)
            info["off"].append(col_off)
            info["n"].append(n)
            col_off += n // 16
        meta.append(info)
    W_total = col_off

    # --- pass 2: fill tables ---
    cores = []
    for k in range(NCORES):
        key_s, idx_s, rank, counts = percore[k]
        blocks = []
        for info in meta:
            t0, t1 = info["t0"], info["t1"]
            tb = t1 - t0
            lo, hi = t0 * P, t1 * P
            for c in range(4):
                D = info["D"][c]
                table = np.full((tb * D, P), zero_idx(c), np.int64)
                sel = (key_s % 4 == c) & (key_s // 4 >= lo) & (key_s // 4 < hi)
                sl = key_s[sel] // 4
                rr = rank[sel]
                tt = sl // P - t0
                pp = sl % P
                table[tt * D + rr, pp] = idx_s[sel]
                arr16 = table.reshape(-1).reshape(-1, 16).T
                assert table.max() <= 32767
                blocks.append(arr16.astype(np.int16))
        idx_all = np.ascontiguousarray(
            np.tile(np.concatenate(blocks, axis=1), (8, 1))
        )  # [128, W]: 16-partition block replicated for the 8 gpsimd cores
        assert idx_all.shape == (128, W_total)

        slot_node = np.full(SLOTS, -1, np.int64)
        mine = node_core == k
        slot_node[node_slot[mine]] = np.nonzero(mine)[0]
        dis_slot = np.zeros(SLOTS, np.float32)
        real = slot_node >= 0
        dis_slot[real] = dis[slot_node[real]]
        dis_pt = np.ascontiguousarray(dis_slot.reshape(TILES, P).T)  # [128, TILES]
        cores.append(
            dict(
                idx_all=idx_all,
                slot_node=slot_node,
                dis_pt=dis_pt,
                dis2_pt=dis_pt * dis_pt,
            )
        )

    return dict(
        cores=cores,
        meta=meta,
        node_core=node_core,
        node_slot=node_slot,
        dis=dis,
        W_total=W_total,
        t_batch=t_batch,
    )



"""Bass/Tile program for the sharded MixHop kernel (one SPMD program, 8 cores)."""

import concourse.bacc as bacc
import concourse.bass as bass
import concourse.tile as tile
from concourse import mybir
from concourse.masks import make_identity


F32 = mybir.dt.float32
I16 = mybir.dt.int16
AX = mybir.AxisListType
OP = mybir.AluOpType
ACT = mybir.ActivationFunctionType



def _ln(nc, sb, h, F, g_rep, be_rep, tag):
    """LayerNorm over free dim F of h [128, F] (in place).  Returns h."""
    mu = sb.tile([P, 1], F32, tag=f"{tag}_mu")
    nc.vector.tensor_reduce(mu[:], h[:], axis=AX.X, op=OP.add)
    nc.scalar.mul(mu[:], mu[:], 1.0 / F)
    cent = sb.tile([P, F], F32, tag=f"{tag}_cent")
    nc.vector.tensor_scalar(cent[:], h[:], mu[:], None, op0=OP.subtract)
    sq = sb.tile([P, F], F32, tag=f"{tag}_sq")
    ssum = sb.tile([P, 1], F32, tag=f"{tag}_ss")
    nc.scalar.activation(sq[:], cent[:], ACT.Square, accum_out=ssum[:])
    std = sb.tile([P, 1], F32, tag=f"{tag}_std")
    nc.scalar.activation(std[:], ssum[:], ACT.Sqrt, bias=prep.LN_EPS, scale=1.0 / F)
    rstd = sb.tile([P, 1], F32, tag=f"{tag}_rstd")
    nc.vector.reciprocal(rstd[:], std[:])
    nc.vector.tensor_scalar(cent[:], cent[:], rstd[:], None, op0=OP.mult)
    nc.vector.tensor_tensor(cent[:], cent[:], g_rep, op=OP.mult)
    nc.vector.tensor_tensor(cent[:], cent[:], be_rep, op=OP.add)
    return cent


def build(meta, W_total, n_hw_cores=8, max_hops=None, do_tail=True, do_gather=True, gather_classes=4, do_reduce=True):
    SLOTS, SHARD, TOT, TILES = prep.SLOTS, prep.SHARD, prep.TOT, prep.TILES
    IN_F, OUT_F, POWERS, MAXP = prep.IN_F, prep.OUT_F, prep.POWERS, prep.MAXP
    NC3 = len(POWERS) * HID  # 192

    nc = bacc.Bacc(
        "TRN2",
        target_bir_lowering=False,
        debug=False,
        num_devices=n_hw_cores,
    )

    x_in = nc.dram_tensor("x_core", [SLOTS, IN_F], F32, kind="ExternalInput")
    idx_in = nc.dram_tensor("idx_all", [128, W_total], I16, kind="ExternalInput")
    dis_in = nc.dram_tensor("dis_pt", [P, TILES], F32, kind="ExternalInput")
    dis2_in = nc.dram_tensor("dis2_pt", [P, TILES], F32, kind="ExternalInput")
    w1_in = nc.dram_tensor("W1", [IN_F, HID], F32, kind="ExternalInput")
    wc_in = nc.dram_tensor("Wcp", [HID, NC3], F32, kind="ExternalInput")
    w2_in = nc.dram_tensor("W2", [NC3, OUT_F], F32, kind="ExternalInput")
    # row constants, each replicated to 128 partitions:
    # b1[64] g1[64] be1[64] bc[192] g2[192] be2[192] b2[40]
    CV = HID * 3 + NC3 * 3 + OUT_F
    cvec_in = nc.dram_tensor("cvec", [P, CV], F32, kind="ExternalInput")
    out_t = nc.dram_tensor("out", [SLOTS, OUT_F], F32, kind="ExternalOutput")

    # internal DRAM state
    u_shard = [
        nc.dram_tensor(f"u_shard{i}", [SHARD, HID], F32, kind="Internal")
        for i in range(2)
    ]
    u_full = [
        nc.dram_tensor(f"u_full{i}", [TOT, HID], F32, kind="Internal")
        for i in range(2)
    ]
    s_save = {
        j: nc.dram_tensor(f"s_save{j}", [SLOTS, HID], F32, kind="Internal")
        for j in POWERS
    }

    rg = [list(range(n_hw_cores))]

    with tile.TileContext(nc) as tc:
        with tc.tile_pool(name="consts", bufs=1) as consts, \
             tc.tile_pool(name="sb", bufs=3) as sb, \
             tc.tile_pool(name="gat", bufs=2) as gat, \
             tc.tile_pool(name="ps", bufs=2, space="PSUM") as ps:
            # ---------------- constants ----------------
            zero_s = consts.tile([P, 1], F32)
            nc.vector.memset(zero_s[:], 0.0)
            nc.const_aps.aps[(F32, 0.0)] = zero_s[:]
            eps_s = consts.tile([P, 1], F32)
            nc.vector.memset(eps_s[:], prep.LN_EPS)
            nc.const_aps.aps[(F32, prep.LN_EPS)] = eps_s[:]
            ident = consts.tile([P, P], F32)
            make_identity(nc, ident[:])
            w1_sb = consts.tile([IN_F, HID], F32)
            nc.sync.dma_start(w1_sb[:], w1_in[:])
            wc_sb = consts.tile([HID, NC3], F32)
            nc.sync.dma_start(wc_sb[:], wc_in[:])
            w2a_sb = consts.tile([P, OUT_F], F32)
            nc.sync.dma_start(w2a_sb[:], w2_in[0:P, :])
            w2b_sb = consts.tile([NC3 - P, OUT_F], F32)
            nc.sync.dma_start(w2b_sb[:], w2_in[P:NC3, :])
            cvec = consts.tile([P, CV], F32)
            nc.sync.dma_start(cvec[:], cvec_in[:])
            o = [0, HID, 2 * HID, 3 * HID, 3 * HID + NC3, 3 * HID + 2 * NC3,
                 3 * HID + 3 * NC3, CV]
            b1_rep = cvec[:, o[0]:o[1]]
            g1_rep = cvec[:, o[1]:o[2]]
            be1_rep = cvec[:, o[2]:o[3]]
            bc_rep = cvec[:, o[3]:o[4]]
            g2_rep = cvec[:, o[4]:o[5]]
            be2_rep = cvec[:, o[5]:o[6]]
            b2_rep = cvec[:, o[6]:o[7]]
            idx_res = consts.tile([P, W_total], I16)
            nc.sync.dma_start(idx_res[:], idx_in[:])
            dis_sb = consts.tile([P, TILES], F32)
            nc.sync.dma_start(dis_sb[:], dis_in[:])
            dis2_sb = consts.tile([P, TILES], F32)
            nc.sync.dma_start(dis2_sb[:], dis2_in[:])
            zrow = consts.tile([1, HID], F32)
            nc.vector.memset(zrow[:], 0.0)
            for i in range(2):
                nc.sync.dma_start(u_shard[i][SLOTS:SHARD, :], zrow[:])

            # ---------------- head: t0 = dis * LN(gelu(x@W1+b1)) ----------------
            for tt in range(TILES):
                xt = sb.tile([P, IN_F], F32, tag="xt")
                nc.sync.dma_start(xt[:], x_in[tt * P:(tt + 1) * P, :])
                tp_ps = ps.tile([P, P], F32, tag="tp")
                nc.tensor.transpose(tp_ps[:], xt[:], ident[:])
                xT = sb.tile([P, P], F32, tag="xT")
                nc.vector.tensor_copy(xT[:], tp_ps[:])
                h_ps = ps.tile([P, HID], F32, tag="mm")
                nc.tensor.matmul(h_ps[:], lhsT=xT[:], rhs=w1_sb[:], start=True, stop=True)
                h = sb.tile([P, HID], F32, tag="h")
                nc.vector.tensor_tensor(h[:], h_ps[:], b1_rep, op=OP.add)
                nc.scalar.activation(h[:], h[:], ACT.Gelu)
                hn = _ln(nc, sb, h, HID, g1_rep, be1_rep, "ln1")
                t0s = sb.tile([P, HID], F32, tag="t0")
                nc.vector.tensor_scalar(t0s[:], hn[:], dis_sb[:, tt:tt + 1], None, op0=OP.mult)
                nc.sync.dma_start(u_shard[0][tt * P:(tt + 1) * P, :], t0s[:])

            # ---------------- hops ----------------
            for hop in range(1, (MAXP if max_hops is None else max_hops) + 1):
                u_in = u_shard[(hop - 1) % 2]
                u_out = u_shard[hop % 2]
                uf = u_full[(hop - 1) % 2]
                if n_hw_cores > 1:
                    nc.gpsimd.collective_compute(
                        "AllGather",
                        OP.bypass,
                        replica_groups=rg,
                        ins=[u_in[:].opt()],
                        outs=[uf[:].opt()],
                    )
                else:
                    nc.sync.dma_start(uf[0:SHARD, :], u_in[:])
                uf4 = uf[:].rearrange("(a b) f -> a (b f)", b=4)
                for info in meta:
                    if not do_gather:
                        break
                    t0, t1 = info["t0"], info["t1"]
                    tb = t1 - t0
                    Gs = []
                    for c in range(gather_classes):
                        D, off, n = info["D"][c], info["off"][c], info["n"][c]
                        G = gat.tile([P, tb * D * HID], F32, tag=f"g{c}")
                        nc.gpsimd.dma_gather(
                            G[:].rearrange("p (c f) -> p c f", f=HID),
                            uf4[:, c * HID:(c + 1) * HID],
                            idx_res[:, off:off + n // 16],
                            n,
                            n,
                            HID,
                            elem_step=4 * HID,
                            single_packet=False,
                        )
                        Gs.append((G[:], D))
                    self_sb = sb.tile([P, tb * HID], F32, tag="selft")
                    nc.sync.dma_start(
                        self_sb[:].rearrange("p (t f) -> p t f", f=HID),
                        u_in[t0 * P:t1 * P, :].rearrange("(t p) f -> p t f", p=P),
                    )
                    acc = sb.tile([P, tb * HID], F32, tag="acc")
                    tmp = sb.tile([P, tb * HID], F32, tag="rtmp")
                    if not do_reduce:
                        nc.vector.tensor_copy(acc[:], self_sb[:])
                    for c in range(gather_classes if do_reduce else 0):
                        gslice, D = Gs[c]
                        gv = gslice.rearrange("p (t d f) -> p t f d", t=tb, d=D, f=HID)
                        dst = acc if c == 0 else tmp
                        nc.vector.tensor_reduce(dst[:], gv, axis=AX.X, op=OP.add)
                        if c > 0:
                            nc.vector.tensor_tensor(acc[:], acc[:], tmp[:], op=OP.add)
                    nc.vector.tensor_tensor(acc[:], acc[:], self_sb[:], op=OP.add)
                    if hop in POWERS:
                        nc.sync.dma_start(
                            s_save[hop][t0 * P:t1 * P, :].rearrange(
                                "(t p) f -> p t f", p=P
                            ),
                            acc[:].rearrange("p (t f) -> p t f", f=HID),
                        )
                    t_new = sb.tile([P, tb * HID], F32, tag="tnew")
                    for t in range(tb):
                        nc.vector.tensor_scalar(
                            t_new[:, t * HID:(t + 1) * HID],
                            acc[:, t * HID:(t + 1) * HID],
                            dis2_sb[:, t0 + t:t0 + t + 1],
                            None,
                            op0=OP.mult,
                        )
                    nc.sync.dma_start(
                        u_out[t0 * P:t1 * P, :].rearrange("(t p) f -> p t f", p=P),
                        t_new[:].rearrange("p (t f) -> p t f", f=HID),
                    )

            # ---------------- tail ----------------
            for tt in range(TILES if do_tail else 0):
                scat = sb.tile([P, NC3], F32, tag="scat")
                for ji, j in enumerate(POWERS):
                    nc.sync.dma_start(
                        scat[:, ji * HID:(ji + 1) * HID],
                        s_save[j][tt * P:(tt + 1) * P, :],
                    )
                nc.vector.tensor_scalar(
                    scat[:], scat[:], dis_sb[:, tt:tt + 1], None, op0=OP.mult
                )
                # transpose each power's 64 cols separately (base partition 0)
                mm_ps = ps.tile([P, NC3], F32, tag="mm")
                for ji in range(3):
                    tpj = ps.tile([HID, P], F32, tag="tp")
                    nc.tensor.transpose(tpj[:], scat[:, ji * HID:(ji + 1) * HID], ident[:])
                    sTj = sb.tile([HID, P], F32, tag="sTj")
                    nc.vector.tensor_copy(sTj[:], tpj[:])
                    nc.tensor.matmul(mm_ps[:, ji * HID:(ji + 1) * HID], lhsT=sTj[:],
                                     rhs=wc_sb[:, ji * HID:(ji + 1) * HID],
                                     start=True, stop=True)
                hc = sb.tile([P, NC3], F32, tag="hc")
                nc.vector.tensor_tensor(hc[:], mm_ps[:], bc_rep, op=OP.add)
                nc.scalar.activation(hc[:], hc[:], ACT.Gelu)
                hn = _ln(nc, sb, hc, NC3, g2_rep, be2_rep, "ln2")
                tpc = ps.tile([P, P], F32, tag="tp")
                nc.tensor.transpose(tpc[:], hn[:, 0:P], ident[:])
                hTa = sb.tile([P, P], F32, tag="hTa")
                nc.vector.tensor_copy(hTa[:], tpc[:])
                tpd = ps.tile([NC3 - P, P], F32, tag="tp")
                nc.tensor.transpose(tpd[:], hn[:, P:NC3], ident[:])
                hTb = sb.tile([NC3 - P, P], F32, tag="hTb")
                nc.vector.tensor_copy(hTb[:], tpd[:])
                o_ps = ps.tile([P, OUT_F], F32, tag="mm")
                nc.tensor.matmul(o_ps[:], lhsT=hTa[:], rhs=w2a_sb[:], start=True, stop=False)
                nc.tensor.matmul(o_ps[:], lhsT=hTb[:], rhs=w2b_sb[:], start=False, stop=True)
                ot = sb.tile([P, OUT_F], F32, tag="ot")
                nc.vector.tensor_tensor(ot[:], o_ps[:], b2_rep, op=OP.add)
                nc.sync.dma_start(out_t[tt * P:(tt + 1) * P, :], ot[:])

    nc.compile()
    return nc


def make_in_maps(inputs, pp):
    """Build the 8 per-core input dicts."""
    POWERS = prep.POWERS
    x = np.asarray(inputs["x"], np.float32)
    Wc = np.asarray(inputs["Wc"], np.float32)
    bc = np.asarray(inputs["bc"], np.float32)
    wcp = np.concatenate([Wc[j] for j in POWERS], axis=1)  # [64, 192]
    bccat = np.concatenate([bc[j] for j in POWERS], axis=0)  # [192]
    cv = np.concatenate([
        np.asarray(inputs["b1"], np.float32),
        np.asarray(inputs["g1"], np.float32),
        np.asarray(inputs["be1"], np.float32),
        bccat,
        np.asarray(inputs["g2"], np.float32),
        np.asarray(inputs["be2"], np.float32),
        np.asarray(inputs["b2"], np.float32),
    ])
    cvec = np.tile(cv[None, :], (P, 1)).copy()

    in_maps = []
    for k in range(prep.NCORES):
        ck = pp["cores"][k]
        sn = ck["slot_node"]
        xk = np.zeros((prep.SLOTS, prep.IN_F), np.float32)
        xk[sn >= 0] = x[sn[sn >= 0]]
        in_maps.append(
            dict(
                x_core=xk,
                idx_all=ck["idx_all"],
                dis_pt=ck["dis_pt"],
                dis2_pt=ck["dis2_pt"],
                W1=np.asarray(inputs["W1"], np.float32),
                Wcp=np.ascontiguousarray(wcp),
                W2=np.asarray(inputs["W2"], np.float32),
                cvec=cvec,
            )
        )
    return in_maps


def assemble_output(results, pp):
    """results: list of per-core dicts with 'out' -> full [N, OUT_F]."""
    out = np.zeros((prep.N, prep.OUT_F), np.float32)
    for k in range(prep.NCORES):
        sn = pp["cores"][k]["slot_node"]
        o = results[k]["out"]
        out[sn[sn >= 0]] = o[sn >= 0]
    return out


_BUILD_CACHE = {}


def kernel(**inputs):
    pp = preprocess(inputs["edge_index"], t_batch=144)
    key = (pp["W_total"], tuple(tuple(i["D"]) for i in pp["meta"]))
    nc = _BUILD_CACHE.get(key)
    if nc is None:
        nc = build(pp["meta"], pp["W_total"])
        _BUILD_CACHE[key] = nc
    in_maps = make_in_maps(inputs, pp)
    from concourse import bass_utils

    res = bass_utils.run_bass_kernel_spmd(nc, in_maps, core_ids=list(range(8)))
    return assemble_output(res.results, pp)


# revision 2
# speedup vs baseline: 1.0723x; 1.0723x over previous
"""Self-contained MixHop Trainium2 kernel: 8-core SPMD Bass program.

kernel(**inputs) takes the FULL inputs (as produced by setup_inputs) and
returns the FULL [100000, 40] float32 output.  Internally: nodes are sharded
across 8 NeuronCores (degree-sorted, class-colored and count-profile retiled
for gather efficiency); each GCN hop is dma_gather + segmented reduce +
per-row scaling with a per-hop AllGather of the 3.2MB shard state; the
per-power tap matmuls run inside the DMA-bound hops; dense lin/LN/GELU stay
data-parallel per core.
"""

import sys

import numpy as np

# the merged preprocessing code below was written against `prep.X`
prep = sys.modules[__name__]

"""Host-side preprocessing for the MixHop Trainium kernel.

Design:
- GCN norm is separable: norm_ij = dis[i]*dis[j].  Keep state t_k = dis * s_k.
  Then s_{k+1} = dis * ((A_raw + I) @ t_k)  and  t_{k+1} = dis^2 * ((A_raw+I) @ t_k).
  Each hop is: gather rows of t_k by src, plain segmented sum per dst, scale.
- Nodes are assigned to 8 cores, degree-sorted (desc) and snake-striped for
  balance.  Core k owns 12500 real slots; padded to 12544 = 98*128 slot grid,
  plus one zero row -> SHARD = 12545 rows per core in the all-gathered state.
- Gather uses dma_gather with int16 indices.  To cover 8*12545 = 100360 rows
  with int16, the state is viewed as 4 interleaved "classes" with
  elem_step = 256 floats (1024B): class c covers rows {4*i + c}, idx = i.
  class(node) = (core + slot) % 4, so the within-tile position (mod 4) decides
  a node's class; a greedy coloring balances every dst's in-neighbour classes.
- Edges are bucketed per (dst tile of 128 slots, class).  Within a batch of
  T tiles, each class c is padded to a rectangle [128, T, D_bc] where
  D_bc = max class-c in-degree over the batch ACROSS ALL CORES (SPMD: one
  program, shared shapes); pad entries point at a zero row of matching class.
- Self loops are NOT in the tables; the device adds the dst's own t_k row
  (loaded contiguously from the local shard) after the class reduction.
"""

import numpy as np

NCORES = 8
P = 128
N = 100000
IN_F = 128
HID = 64
OUT_F = 40
POWERS = [6, 8, 10]
MAXP = 10
LN_EPS = 1e-5

TILES = 98                 # per-core dst tiles
SLOTS = TILES * P          # 12544 padded slots per core
REAL = N // NCORES         # 12500 real nodes per core
SHARD = SLOTS + 1          # + 1 zero row = 12545  (SHARD % 4 == 1)
TOT = NCORES * SHARD       # 100360 all-gathered rows
ZROW = SLOTS               # zero row slot within a shard


def set_dims(n, tiles):
    """Reconfigure for a smaller debug instance (keeps NCORES/feature dims)."""
    global N, TILES, SLOTS, REAL, SHARD, TOT, ZROW
    N = n
    TILES = tiles
    SLOTS = TILES * P
    REAL = N // NCORES
    assert REAL * NCORES == N and REAL <= SLOTS
    SHARD = SLOTS + 1
    TOT = NCORES * SHARD
    ZROW = SLOTS


def zero_idx(c):
    """Gather index of a zero row for class c (row SHARD*c + SLOTS)."""
    z = SHARD * c + SLOTS
    assert z % 4 == c, (c, z)
    return z // 4


def _assign_nodes(deg):
    """Degree-sorted snake assignment: node -> (core, tile, degree rank)."""
    order = np.argsort(-deg, kind="stable")
    node_core = np.empty(N, np.int32)
    node_rank = np.empty(N, np.int32)  # rank within core (= provisional slot)
    snake = np.tile(
        np.concatenate([np.arange(NCORES), np.arange(NCORES - 1, -1, -1)]),
        N // (2 * NCORES) + 1,
    )[:N]
    node_core[order] = snake
    node_rank[order] = np.arange(N) // NCORES
    return node_core, node_rank


def _color_nodes(src, dst, node_core, node_tile, out_deg_order):
    """Greedy 4-coloring balancing each dst's in-neighbour colors, subject to
    32-per-color capacity within each (core, tile)."""
    cnt = np.zeros((N, 4), np.int32)
    cap = np.full((NCORES, TILES, 4), 32, np.int32)
    color = np.full(N, -1, np.int8)

    # CSR of out-neighbours by src
    order = np.argsort(src, kind="stable")
    dst_s = dst[order]
    indptr = np.zeros(N + 1, np.int64)
    np.cumsum(np.bincount(src, minlength=N), out=indptr[1:])

    big = np.int32(1 << 20)
    for n in out_deg_order:
        k, t = node_core[n], node_tile[n]
        capn = cap[k, t]
        ds = dst_s[indptr[n] : indptr[n + 1]]
        if len(ds):
            score = cnt[ds].sum(axis=0, dtype=np.int64)
        else:
            score = np.zeros(4, np.int64)
        score = np.where(capn > 0, score, big)
        c = int(np.argmin(score))
        color[n] = c
        capn[c] -= 1
        if len(ds):
            np.add.at(cnt, (ds, c), 1)

    # refinement sweeps: move a node to a color that lowers sum of per-dst maxima
    rng = np.random.default_rng(7)
    for _ in range(2):
        order = rng.permutation(N)
        for n in order:
            ds = dst_s[indptr[n] : indptr[n + 1]]
            if not len(ds):
                continue
            k, t = node_core[n], node_tile[n]
            capn = cap[k, t]
            c_old = color[n]
            sub = cnt[ds]  # [deg, 4]
            mx = sub.max(axis=1)
            # cost delta of removing from c_old: -1 where c_old was unique max
            col_old = sub[:, c_old]
            unique_max = (col_old == mx) & ((sub == mx[:, None]).sum(axis=1) == 1)
            gain_remove = int(unique_max.sum())
            best_c, best_delta = c_old, 0
            for c_new in range(4):
                if c_new == c_old or capn[c_new] <= 0:
                    continue
                add_cost = int((sub[:, c_new] + 1 > mx).sum())
                delta = add_cost - gain_remove
                if delta < best_delta:
                    best_delta, best_c = delta, c_new
            if best_c != c_old:
                np.add.at(cnt, (ds, c_old), -1)
                np.add.at(cnt, (ds, best_c), 1)
                capn[c_old] += 1
                capn[best_c] -= 1
                color[n] = best_c
    return color, cnt


def preprocess(edge_index, t_batch=6, color=True):
    """edge_index: [2, E].  Returns dict with shared geometry + per-core tables."""
    src = np.asarray(edge_index[0], dtype=np.int64)
    dst = np.asarray(edge_index[1], dtype=np.int64)

    deg = np.bincount(dst, minlength=N).astype(np.float64) + 1.0  # + self loop
    dis = (1.0 / np.sqrt(deg)).astype(np.float32)

    node_core, node_rank = _assign_nodes(deg)
    node_tile = node_rank // P

    # --- choose within-tile positions (the mod-4 position fixes the class) ---
    if color:
        out_deg = np.bincount(src, minlength=N)
        odo = np.argsort(-out_deg, kind="stable")
        col, cnt = _color_nodes(src, dst, node_core, node_tile, odo)
    else:
        col = ((node_core + node_rank) % 4).astype(np.int8)
        key0 = dst * 4 + col[src].astype(np.int64)
        cnt = np.bincount(key0, minlength=N * 4).reshape(N, 4).astype(np.int32)

    # Re-tile: per core, per color, sort dsts by their pooled cost (max class
    # in-count) descending and chunk into 32s -> tile = chunk index.  This
    # makes every tile homogeneous in the quantity the rectangles pool over,
    # while the 32-per-color capacity holds by construction.
    max_cnt = cnt.max(axis=1)
    node_slot = np.empty(N, np.int64)
    for k in range(NCORES):
        idx = np.nonzero(node_core == k)[0]
        plist = {c: np.nonzero((k + np.arange(P)) % 4 == c)[0] for c in range(4)}
        for c in range(4):
            nodes_c = idx[col[idx] == c]
            order_c = nodes_c[np.argsort(-max_cnt[nodes_c], kind="stable")]
            t_of = np.arange(len(order_c)) // 32
            r_of = np.arange(len(order_c)) % 32
            assert t_of.max() < TILES
            node_slot[order_c] = t_of * P + plist[c][r_of]

    abs_row = node_core.astype(np.int64) * SHARD + node_slot
    cls_of = (abs_row % 4).astype(np.int64)
    if color:
        assert (cls_of == col.astype(np.int64)).all()

    # --- per-core edge grouping, pass 1: counts for shared geometry ---
    percore = []
    for k in range(NCORES):
        m = node_core[dst] == k
        s_k = src[m]
        d_slot = node_slot[dst[m]]
        e_cls = cls_of[s_k]
        key = d_slot * 4 + e_cls
        sort = np.argsort(key, kind="stable")
        key_s = key[sort]
        idx_s = ((abs_row[s_k] - e_cls) // 4)[sort]
        counts = np.bincount(key_s, minlength=SLOTS * 4).reshape(SLOTS, 4)
        starts = np.concatenate([[0], np.cumsum(counts.reshape(-1))[:-1]])
        rank = np.arange(len(key_s)) - starts[key_s]
        percore.append((key_s, idx_s, rank, counts))

    # per-(tile, class) shared D (max over cores)
    tileD = np.zeros((TILES, 4), np.int64)
    for t in range(TILES):
        lo, hi = t * P, (t + 1) * P
        for c in range(4):
            tileD[t, c] = max(
                max(int(percore[k][3][lo:hi, c].max()) for k in range(NCORES)), 1
            )

    # --- batches: pack tiles while tb * sum_c(max D_c) <= budget cols ---
    budget = t_batch  # interpreted as gather-column budget when > 32
    if budget <= 32:
        budget = 224
    batches = []
    t0 = 0
    while t0 < TILES:
        t1 = t0 + 1
        while t1 < TILES and t1 - t0 < 8:
            Dk = tileD[t0:t1 + 1].max(axis=0)
            if (t1 + 1 - t0) * int(Dk.sum()) > budget:
                break
            t1 += 1
        batches.append((t0, t1))
        t0 = t1

    meta = []
    col_off = 0
    for (t0, t1) in batches:
        tb = t1 - t0
        info = {"t0": t0, "t1": t1, "D": [], "off": [], "n": []}
        for c in range(4):
            D = int(tileD[t0:t1, c].max())
            n = tb * D * P
            info["D"].append(D)
            info["off"].append(col_off)
            info["n"].append(n)
            col_off += n // 16
        meta.append(info)
    W_total = col_off

    # --- pass 2: fill tables ---
    cores = []
    for k in range(NCORES):
        key_s, idx_s, rank, counts = percore[k]
        blocks = []
        for info in meta:
            t0, t1 = info["t0"], info["t1"]
            tb = t1 - t0
            lo, hi = t0 * P, t1 * P
            for c in range(4):
                D = info["D"][c]
                table = np.full((tb * D, P), zero_idx(c), np.int64)
                sel = (key_s % 4 == c) & (key_s // 4 >= lo) & (key_s // 4 < hi)
                sl = key_s[sel] // 4
                rr = rank[sel]
                tt = sl // P - t0
                pp = sl % P
                table[tt * D + rr, pp] = idx_s[sel]
                arr16 = table.reshape(-1).reshape(-1, 16).T
                assert table.max() <= 32767
                blocks.append(arr16.astype(np.int16))
        idx_all = np.ascontiguousarray(
            np.tile(np.concatenate(blocks, axis=1), (8, 1))
        )  # [128, W]: 16-partition block replicated for the 8 gpsimd cores
        assert idx_all.shape == (128, W_total)

        slot_node = np.full(SLOTS, -1, np.int64)
        mine = node_core == k
        slot_node[node_slot[mine]] = np.nonzero(mine)[0]
        dis_slot = np.zeros(SLOTS, np.float32)
        real = slot_node >= 0
        dis_slot[real] = dis[slot_node[real]]
        dis_pt = np.ascontiguousarray(dis_slot.reshape(TILES, P).T)  # [128, TILES]
        cores.append(
            dict(
                idx_all=idx_all,
                slot_node=slot_node,
                dis_pt=dis_pt,
                dis2_pt=dis_pt * dis_pt,
            )
        )

    return dict(
        cores=cores,
        meta=meta,
        node_core=node_core,
        node_slot=node_slot,
        dis=dis,
        W_total=W_total,
        t_batch=t_batch,
    )



"""Bass/Tile program for the sharded MixHop kernel (one SPMD program, 8 cores)."""

import concourse.bacc as bacc
import concourse.bass as bass
import concourse.tile as tile
from concourse import mybir
from concourse.masks import make_identity


F32 = mybir.dt.float32
I16 = mybir.dt.int16
AX = mybir.AxisListType
OP = mybir.AluOpType
ACT = mybir.ActivationFunctionType



def _ln(nc, sb, h, F, g_rep, be_rep, tag):
    """LayerNorm over free dim F of h [128, F] (in place).  Returns h."""
    mu = sb.tile([P, 1], F32, tag=f"{tag}_mu")
    nc.vector.tensor_reduce(mu[:], h[:], axis=AX.X, op=OP.add)
    nc.scalar.mul(mu[:], mu[:], 1.0 / F)
    cent = sb.tile([P, F], F32, tag=f"{tag}_cent")
    nc.vector.tensor_scalar(cent[:], h[:], mu[:], None, op0=OP.subtract)
    sq = sb.tile([P, F], F32, tag=f"{tag}_sq")
    ssum = sb.tile([P, 1], F32, tag=f"{tag}_ss")
    nc.scalar.activation(sq[:], cent[:], ACT.Square, accum_out=ssum[:])
    std = sb.tile([P, 1], F32, tag=f"{tag}_std")
    nc.scalar.activation(std[:], ssum[:], ACT.Sqrt, bias=prep.LN_EPS, scale=1.0 / F)
    rstd = sb.tile([P, 1], F32, tag=f"{tag}_rstd")
    nc.vector.reciprocal(rstd[:], std[:])
    nc.vector.tensor_scalar(cent[:], cent[:], rstd[:], None, op0=OP.mult)
    nc.vector.tensor_tensor(cent[:], cent[:], g_rep, op=OP.mult)
    nc.vector.tensor_tensor(cent[:], cent[:], be_rep, op=OP.add)
    return cent


def build(meta, W_total, n_hw_cores=8, max_hops=None, do_tail=True, do_gather=True, gather_classes=4, do_reduce=True):
    SLOTS, SHARD, TOT, TILES = prep.SLOTS, prep.SHARD, prep.TOT, prep.TILES
    IN_F, OUT_F, POWERS, MAXP = prep.IN_F, prep.OUT_F, prep.POWERS, prep.MAXP
    NC3 = len(POWERS) * HID  # 192

    nc = bacc.Bacc(
        "TRN2",
        target_bir_lowering=False,
        debug=False,
        num_devices=n_hw_cores,
    )

    x_in = nc.dram_tensor("x_core", [SLOTS, IN_F], F32, kind="ExternalInput")
    idx_in = nc.dram_tensor("idx_all", [128, W_total], I16, kind="ExternalInput")
    dis_in = nc.dram_tensor("dis_pt", [P, TILES], F32, kind="ExternalInput")
    dis2_in = nc.dram_tensor("dis2_pt", [P, TILES], F32, kind="ExternalInput")
    w1_in = nc.dram_tensor("W1", [IN_F, HID], F32, kind="ExternalInput")
    wc_in = nc.dram_tensor("Wcp", [HID, NC3], F32, kind="ExternalInput")
    w2_in = nc.dram_tensor("W2", [NC3, OUT_F], F32, kind="ExternalInput")
    # row constants, each replicated to 128 partitions:
    # b1[64] g1[64] be1[64] bc[192] g2[192] be2[192] b2[40]
    CV = HID * 3 + NC3 * 3 + OUT_F
    cvec_in = nc.dram_tensor("cvec", [P, CV], F32, kind="ExternalInput")
    out_t = nc.dram_tensor("out", [SLOTS, OUT_F], F32, kind="ExternalOutput")

    # internal DRAM state
    u_shard = [
        nc.dram_tensor(f"u_shard{i}", [SHARD, HID], F32, kind="Internal")
        for i in range(2)
    ]
    u_full = [
        nc.dram_tensor(f"u_full{i}", [TOT, HID], F32, kind="Internal")
        for i in range(2)
    ]
    s_save = {
        j: nc.dram_tensor(f"s_save{j}", [SLOTS, HID], F32, kind="Internal")
        for j in POWERS
    }

    rg = [list(range(n_hw_cores))]

    with tile.TileContext(nc) as tc:
        with tc.tile_pool(name="consts", bufs=1) as consts, \
             tc.tile_pool(name="sb", bufs=2) as sb, \
             tc.tile_pool(name="gat", bufs=2) as gat, \
             tc.tile_pool(name="ps", bufs=2, space="PSUM") as ps:
            # ---------------- constants ----------------
            zero_s = consts.tile([P, 1], F32)
            nc.vector.memset(zero_s[:], 0.0)
            nc.const_aps.aps[(F32, 0.0)] = zero_s[:]
            eps_s = consts.tile([P, 1], F32)
            nc.vector.memset(eps_s[:], prep.LN_EPS)
            nc.const_aps.aps[(F32, prep.LN_EPS)] = eps_s[:]
            ident = consts.tile([P, P], F32)
            make_identity(nc, ident[:])
            w1_sb = consts.tile([IN_F, HID], F32)
            nc.sync.dma_start(w1_sb[:], w1_in[:])
            wc_sb = consts.tile([HID, NC3], F32)
            nc.sync.dma_start(wc_sb[:], wc_in[:])
            w2a_sb = consts.tile([P, OUT_F], F32)
            nc.sync.dma_start(w2a_sb[:], w2_in[0:P, :])
            w2b_sb = consts.tile([NC3 - P, OUT_F], F32)
            nc.sync.dma_start(w2b_sb[:], w2_in[P:NC3, :])
            cvec = consts.tile([P, CV], F32)
            nc.sync.dma_start(cvec[:], cvec_in[:])
            o = [0, HID, 2 * HID, 3 * HID, 3 * HID + NC3, 3 * HID + 2 * NC3,
                 3 * HID + 3 * NC3, CV]
            b1_rep = cvec[:, o[0]:o[1]]
            g1_rep = cvec[:, o[1]:o[2]]
            be1_rep = cvec[:, o[2]:o[3]]
            bc_rep = cvec[:, o[3]:o[4]]
            g2_rep = cvec[:, o[4]:o[5]]
            be2_rep = cvec[:, o[5]:o[6]]
            b2_rep = cvec[:, o[6]:o[7]]
            idx_res = consts.tile([P, W_total], I16)
            nc.sync.dma_start(idx_res[:], idx_in[:])
            dis_sb = consts.tile([P, TILES], F32)
            nc.sync.dma_start(dis_sb[:], dis_in[:])
            dis2_sb = consts.tile([P, TILES], F32)
            nc.sync.dma_start(dis2_sb[:], dis2_in[:])
            zrow = consts.tile([1, HID], F32)
            nc.vector.memset(zrow[:], 0.0)
            for i in range(2):
                nc.sync.dma_start(u_shard[i][SLOTS:SHARD, :], zrow[:])

            # ---------------- head: t0 = dis * LN(gelu(x@W1+b1)) ----------------
            for tt in range(TILES):
                xt = sb.tile([P, IN_F], F32, tag="xt")
                nc.sync.dma_start(xt[:], x_in[tt * P:(tt + 1) * P, :])
                tp_ps = ps.tile([P, P], F32, tag="tp")
                nc.tensor.transpose(tp_ps[:], xt[:], ident[:])
                xT = sb.tile([P, P], F32, tag="xT")
                nc.vector.tensor_copy(xT[:], tp_ps[:])
                h_ps = ps.tile([P, HID], F32, tag="mm")
                nc.tensor.matmul(h_ps[:], lhsT=xT[:], rhs=w1_sb[:], start=True, stop=True)
                h = sb.tile([P, HID], F32, tag="h")
                nc.vector.tensor_tensor(h[:], h_ps[:], b1_rep, op=OP.add)
                nc.scalar.activation(h[:], h[:], ACT.Gelu)
                hn = _ln(nc, sb, h, HID, g1_rep, be1_rep, "ln1")
                t0s = sb.tile([P, HID], F32, tag="t0")
                nc.vector.tensor_scalar(t0s[:], hn[:], dis_sb[:, tt:tt + 1], None, op0=OP.mult)
                nc.sync.dma_start(u_shard[0][tt * P:(tt + 1) * P, :], t0s[:])

            # ---------------- hops ----------------
            for hop in range(1, (MAXP if max_hops is None else max_hops) + 1):
                u_in = u_shard[(hop - 1) % 2]
                u_out = u_shard[hop % 2]
                uf = u_full[(hop - 1) % 2]
                if n_hw_cores > 1:
                    nc.gpsimd.collective_compute(
                        "AllGather",
                        OP.bypass,
                        replica_groups=rg,
                        ins=[u_in[:].opt()],
                        outs=[uf[:].opt()],
                    )
                else:
                    nc.sync.dma_start(uf[0:SHARD, :], u_in[:])
                uf4 = uf[:].rearrange("(a b) f -> a (b f)", b=4)
                for info in meta:
                    if not do_gather:
                        break
                    t0, t1 = info["t0"], info["t1"]
                    tb = t1 - t0
                    Gs = []
                    for c in range(gather_classes):
                        D, off, n = info["D"][c], info["off"][c], info["n"][c]
                        G = gat.tile([P, tb * D * HID], F32, tag=f"g{c}")
                        nc.gpsimd.dma_gather(
                            G[:].rearrange("p (c f) -> p c f", f=HID),
                            uf4[:, c * HID:(c + 1) * HID],
                            idx_res[:, off:off + n // 16],
                            n,
                            n,
                            HID,
                            elem_step=4 * HID,
                            single_packet=False,
                        )
                        Gs.append((G[:], D))
                    self_sb = sb.tile([P, tb * HID], F32, tag="selft")
                    nc.sync.dma_start(
                        self_sb[:].rearrange("p (t f) -> p t f", f=HID),
                        u_in[t0 * P:t1 * P, :].rearrange("(t p) f -> p t f", p=P),
                    )
                    acc = sb.tile([P, tb * HID], F32, tag="acc")
                    tmp = sb.tile([P, tb * HID], F32, tag="rtmp")
                    if not do_reduce:
                        nc.vector.tensor_copy(acc[:], self_sb[:])
                    for c in range(gather_classes if do_reduce else 0):
                        gslice, D = Gs[c]
                        gv = gslice.rearrange("p (t d f) -> p t f d", t=tb, d=D, f=HID)
                        dst = acc if c == 0 else tmp
                        nc.vector.tensor_reduce(dst[:], gv, axis=AX.X, op=OP.add)
                        if c > 0:
                            nc.vector.tensor_tensor(acc[:], acc[:], tmp[:], op=OP.add)
                    nc.vector.tensor_tensor(acc[:], acc[:], self_sb[:], op=OP.add)
                    if hop in POWERS:
                        # tap: o_j = (dis * acc) @ Wc_j, computed here where the
                        # PE is idle (the hop is DMA-bound); tail just concats.
                        ji = POWERS.index(hop)
                        for t in range(tb):
                            st = sb.tile([P, HID], F32, tag="st")
                            nc.vector.tensor_scalar(
                                st[:], acc[:, t * HID:(t + 1) * HID],
                                dis_sb[:, t0 + t:t0 + t + 1], None, op0=OP.mult,
                            )
                            tpv = ps.tile([HID, P], F32, tag="tp")
                            nc.tensor.transpose(tpv[:], st[:], ident[:])
                            sT = sb.tile([HID, P], F32, tag="sTh")
                            nc.vector.tensor_copy(sT[:], tpv[:])
                            o_ps = ps.tile([P, HID], F32, tag="mm")
                            nc.tensor.matmul(
                                o_ps[:], lhsT=sT[:],
                                rhs=wc_sb[:, ji * HID:(ji + 1) * HID],
                                start=True, stop=True,
                            )
                            osb = sb.tile([P, HID], F32, tag="osb")
                            nc.vector.tensor_copy(osb[:], o_ps[:])
                            nc.sync.dma_start(
                                s_save[hop][(t0 + t) * P:(t0 + t + 1) * P, :],
                                osb[:],
                            )
                    if hop < MAXP:
                        t_new = sb.tile([P, tb * HID], F32, tag="tnew")
                        for t in range(tb):
                            nc.vector.tensor_scalar(
                                t_new[:, t * HID:(t + 1) * HID],
                                acc[:, t * HID:(t + 1) * HID],
                                dis2_sb[:, t0 + t:t0 + t + 1],
                                None,
                                op0=OP.mult,
                            )
                        nc.sync.dma_start(
                            u_out[t0 * P:t1 * P, :].rearrange("(t p) f -> p t f", p=P),
                            t_new[:].rearrange("p (t f) -> p t f", f=HID),
                        )

            # ---------------- tail ----------------
            for tt in range(TILES if do_tail else 0):
                scat = sb.tile([P, NC3], F32, tag="scat")
                for ji, j in enumerate(POWERS):
                    nc.sync.dma_start(
                        scat[:, ji * HID:(ji + 1) * HID],
                        s_save[j][tt * P:(tt + 1) * P, :],
                    )
                hc = sb.tile([P, NC3], F32, tag="hc")
                nc.vector.tensor_tensor(hc[:], scat[:], bc_rep, op=OP.add)
                nc.scalar.activation(hc[:], hc[:], ACT.Gelu)
                hn = _ln(nc, sb, hc, NC3, g2_rep, be2_rep, "ln2")
                tpc = ps.tile([P, P], F32, tag="tp")
                nc.tensor.transpose(tpc[:], hn[:, 0:P], ident[:])
                hTa = sb.tile([P, P], F32, tag="hTa")
                nc.vector.tensor_copy(hTa[:], tpc[:])
                tpd = ps.tile([NC3 - P, P], F32, tag="tp")
                nc.tensor.transpose(tpd[:], hn[:, P:NC3], ident[:])
                hTb = sb.tile([NC3 - P, P], F32, tag="hTb")
                nc.vector.tensor_copy(hTb[:], tpd[:])
                o_ps = ps.tile([P, OUT_F], F32, tag="mm")
                nc.tensor.matmul(o_ps[:], lhsT=hTa[:], rhs=w2a_sb[:], start=True, stop=False)
                nc.tensor.matmul(o_ps[:], lhsT=hTb[:], rhs=w2b_sb[:], start=False, stop=True)
                ot = sb.tile([P, OUT_F], F32, tag="ot")
                nc.vector.tensor_tensor(ot[:], o_ps[:], b2_rep, op=OP.add)
                nc.sync.dma_start(out_t[tt * P:(tt + 1) * P, :], ot[:])

    nc.compile()
    return nc


def make_in_maps(inputs, pp):
    """Build the 8 per-core input dicts."""
    POWERS = prep.POWERS
    x = np.asarray(inputs["x"], np.float32)
    Wc = np.asarray(inputs["Wc"], np.float32)
    bc = np.asarray(inputs["bc"], np.float32)
    wcp = np.concatenate([Wc[j] for j in POWERS], axis=1)  # [64, 192]
    bccat = np.concatenate([bc[j] for j in POWERS], axis=0)  # [192]
    cv = np.concatenate([
        np.asarray(inputs["b1"], np.float32),
        np.asarray(inputs["g1"], np.float32),
        np.asarray(inputs["be1"], np.float32),
        bccat,
        np.asarray(inputs["g2"], np.float32),
        np.asarray(inputs["be2"], np.float32),
        np.asarray(inputs["b2"], np.float32),
    ])
    cvec = np.tile(cv[None, :], (P, 1)).copy()

    in_maps = []
    for k in range(prep.NCORES):
        ck = pp["cores"][k]
        sn = ck["slot_node"]
        xk = np.zeros((prep.SLOTS, prep.IN_F), np.float32)
        xk[sn >= 0] = x[sn[sn >= 0]]
        in_maps.append(
            dict(
                x_core=xk,
                idx_all=ck["idx_all"],
                dis_pt=ck["dis_pt"],
                dis2_pt=ck["dis2_pt"],
                W1=np.asarray(inputs["W1"], np.float32),
                Wcp=np.ascontiguousarray(wcp),
                W2=np.asarray(inputs["W2"], np.float32),
                cvec=cvec,
            )
        )
    return in_maps


def assemble_output(results, pp):
    """results: list of per-core dicts with 'out' -> full [N, OUT_F]."""
    out = np.zeros((prep.N, prep.OUT_F), np.float32)
    for k in range(prep.NCORES):
        sn = pp["cores"][k]["slot_node"]
        o = results[k]["out"]
        out[sn[sn >= 0]] = o[sn >= 0]
    return out


_BUILD_CACHE = {}


def kernel(**inputs):
    pp = preprocess(inputs["edge_index"], t_batch=144)
    key = (pp["W_total"], tuple(tuple(i["D"]) for i in pp["meta"]))
    nc = _BUILD_CACHE.get(key)
    if nc is None:
        nc = build(pp["meta"], pp["W_total"])
        _BUILD_CACHE[key] = nc
    in_maps = make_in_maps(inputs, pp)
    from concourse import bass_utils

    res = bass_utils.run_bass_kernel_spmd(nc, in_maps, core_ids=list(range(8)))
    return assemble_output(res.results, pp)


# revision 3
# speedup vs baseline: 1.0938x; 1.0201x over previous
"""Self-contained MixHop Trainium2 kernel: 8-core SPMD Bass program.

kernel(**inputs) takes the FULL inputs (as produced by setup_inputs) and
returns the FULL [100000, 40] float32 output.  Internally: nodes are sharded
across 8 NeuronCores (degree-sorted, class-colored and count-profile retiled
for gather efficiency); each GCN hop is dma_gather + segmented reduce +
per-row scaling with a per-hop AllGather of the 3.2MB shard state; the shard
state stays resident in SBUF for the self-loop term; the per-power tap
matmuls run inside the DMA-bound hops; dense lin/LN/GELU stay data-parallel
per core.
"""

import sys

import numpy as np

# the merged preprocessing code below was written against `prep.X`
prep = sys.modules[__name__]

"""Host-side preprocessing for the MixHop Trainium kernel.

Design:
- GCN norm is separable: norm_ij = dis[i]*dis[j].  Keep state t_k = dis * s_k.
  Then s_{k+1} = dis * ((A_raw + I) @ t_k)  and  t_{k+1} = dis^2 * ((A_raw+I) @ t_k).
  Each hop is: gather rows of t_k by src, plain segmented sum per dst, scale.
- Nodes are assigned to 8 cores, degree-sorted (desc) and snake-striped for
  balance.  Core k owns 12500 real slots; padded to 12544 = 98*128 slot grid,
  plus one zero row -> SHARD = 12545 rows per core in the all-gathered state.
- Gather uses dma_gather with int16 indices.  To cover 8*12545 = 100360 rows
  with int16, the state is viewed as 4 interleaved "classes" with
  elem_step = 256 floats (1024B): class c covers rows {4*i + c}, idx = i.
  class(node) = (core + slot) % 4, so the within-tile position (mod 4) decides
  a node's class; a greedy coloring balances every dst's in-neighbour classes.
- Edges are bucketed per (dst tile of 128 slots, class).  Within a batch of
  T tiles, each class c is padded to a rectangle [128, T, D_bc] where
  D_bc = max class-c in-degree over the batch ACROSS ALL CORES (SPMD: one
  program, shared shapes); pad entries point at a zero row of matching class.
- Self loops are NOT in the tables; the device adds the dst's own t_k row
  (loaded contiguously from the local shard) after the class reduction.
"""

import numpy as np

NCORES = 8
P = 128
N = 100000
IN_F = 128
HID = 64
OUT_F = 40
POWERS = [6, 8, 10]
MAXP = 10
LN_EPS = 1e-5

TILES = 98                 # per-core dst tiles
SLOTS = TILES * P          # 12544 padded slots per core
REAL = N // NCORES         # 12500 real nodes per core
SHARD = SLOTS + 1          # + 1 zero row = 12545  (SHARD % 4 == 1)
TOT = NCORES * SHARD       # 100360 all-gathered rows
ZROW = SLOTS               # zero row slot within a shard


def set_dims(n, tiles):
    """Reconfigure for a smaller debug instance (keeps NCORES/feature dims)."""
    global N, TILES, SLOTS, REAL, SHARD, TOT, ZROW
    N = n
    TILES = tiles
    SLOTS = TILES * P
    REAL = N // NCORES
    assert REAL * NCORES == N and REAL <= SLOTS
    SHARD = SLOTS + 1
    TOT = NCORES * SHARD
    ZROW = SLOTS


def zero_idx(c):
    """Gather index of a zero row for class c (row SHARD*c + SLOTS)."""
    z = SHARD * c + SLOTS
    assert z % 4 == c, (c, z)
    return z // 4


def _assign_nodes(deg):
    """Degree-sorted snake assignment: node -> (core, tile, degree rank)."""
    order = np.argsort(-deg, kind="stable")
    node_core = np.empty(N, np.int32)
    node_rank = np.empty(N, np.int32)  # rank within core (= provisional slot)
    snake = np.tile(
        np.concatenate([np.arange(NCORES), np.arange(NCORES - 1, -1, -1)]),
        N // (2 * NCORES) + 1,
    )[:N]
    node_core[order] = snake
    node_rank[order] = np.arange(N) // NCORES
    return node_core, node_rank


def _color_nodes(src, dst, node_core, node_tile, out_deg_order):
    """Greedy 4-coloring balancing each dst's in-neighbour colors, subject to
    32-per-color capacity within each (core, tile)."""
    cnt = np.zeros((N, 4), np.int32)
    cap = np.full((NCORES, TILES, 4), 32, np.int32)
    color = np.full(N, -1, np.int8)

    # CSR of out-neighbours by src
    order = np.argsort(src, kind="stable")
    dst_s = dst[order]
    indptr = np.zeros(N + 1, np.int64)
    np.cumsum(np.bincount(src, minlength=N), out=indptr[1:])

    big = np.int32(1 << 20)
    for n in out_deg_order:
        k, t = node_core[n], node_tile[n]
        capn = cap[k, t]
        ds = dst_s[indptr[n] : indptr[n + 1]]
        if len(ds):
            score = cnt[ds].sum(axis=0, dtype=np.int64)
        else:
            score = np.zeros(4, np.int64)
        score = np.where(capn > 0, score, big)
        c = int(np.argmin(score))
        color[n] = c
        capn[c] -= 1
        if len(ds):
            np.add.at(cnt, (ds, c), 1)

    # refinement sweeps: move a node to a color that lowers sum of per-dst maxima
    rng = np.random.default_rng(7)
    for _ in range(2):
        order = rng.permutation(N)
        for n in order:
            ds = dst_s[indptr[n] : indptr[n + 1]]
            if not len(ds):
                continue
            k, t = node_core[n], node_tile[n]
            capn = cap[k, t]
            c_old = color[n]
            sub = cnt[ds]  # [deg, 4]
            mx = sub.max(axis=1)
            # cost delta of removing from c_old: -1 where c_old was unique max
            col_old = sub[:, c_old]
            unique_max = (col_old == mx) & ((sub == mx[:, None]).sum(axis=1) == 1)
            gain_remove = int(unique_max.sum())
            best_c, best_delta = c_old, 0
            for c_new in range(4):
                if c_new == c_old or capn[c_new] <= 0:
                    continue
                add_cost = int((sub[:, c_new] + 1 > mx).sum())
                delta = add_cost - gain_remove
                if delta < best_delta:
                    best_delta, best_c = delta, c_new
            if best_c != c_old:
                np.add.at(cnt, (ds, c_old), -1)
                np.add.at(cnt, (ds, best_c), 1)
                capn[c_old] += 1
                capn[best_c] -= 1
                color[n] = best_c
    return color, cnt


def preprocess(edge_index, t_batch=6, color=True):
    """edge_index: [2, E].  Returns dict with shared geometry + per-core tables."""
    src = np.asarray(edge_index[0], dtype=np.int64)
    dst = np.asarray(edge_index[1], dtype=np.int64)

    deg = np.bincount(dst, minlength=N).astype(np.float64) + 1.0  # + self loop
    dis = (1.0 / np.sqrt(deg)).astype(np.float32)

    node_core, node_rank = _assign_nodes(deg)
    node_tile = node_rank // P

    # --- choose within-tile positions (the mod-4 position fixes the class) ---
    if color:
        out_deg = np.bincount(src, minlength=N)
        odo = np.argsort(-out_deg, kind="stable")
        col, cnt = _color_nodes(src, dst, node_core, node_tile, odo)
    else:
        col = ((node_core + node_rank) % 4).astype(np.int8)
        key0 = dst * 4 + col[src].astype(np.int64)
        cnt = np.bincount(key0, minlength=N * 4).reshape(N, 4).astype(np.int32)

    # Re-tile: per core, per color, sort dsts by their pooled cost (max class
    # in-count) descending and chunk into 32s -> tile = chunk index.  This
    # makes every tile homogeneous in the quantity the rectangles pool over,
    # while the 32-per-color capacity holds by construction.
    max_cnt = cnt.max(axis=1)
    node_slot = np.empty(N, np.int64)
    for k in range(NCORES):
        idx = np.nonzero(node_core == k)[0]
        plist = {c: np.nonzero((k + np.arange(P)) % 4 == c)[0] for c in range(4)}
        for c in range(4):
            nodes_c = idx[col[idx] == c]
            order_c = nodes_c[np.argsort(-max_cnt[nodes_c], kind="stable")]
            t_of = np.arange(len(order_c)) // 32
            r_of = np.arange(len(order_c)) % 32
            assert t_of.max() < TILES
            node_slot[order_c] = t_of * P + plist[c][r_of]

    abs_row = node_core.astype(np.int64) * SHARD + node_slot
    cls_of = (abs_row % 4).astype(np.int64)
    if color:
        assert (cls_of == col.astype(np.int64)).all()

    # --- per-core edge grouping, pass 1: counts for shared geometry ---
    percore = []
    for k in range(NCORES):
        m = node_core[dst] == k
        s_k = src[m]
        d_slot = node_slot[dst[m]]
        e_cls = cls_of[s_k]
        key = d_slot * 4 + e_cls
        sort = np.argsort(key, kind="stable")
        key_s = key[sort]
        idx_s = ((abs_row[s_k] - e_cls) // 4)[sort]
        counts = np.bincount(key_s, minlength=SLOTS * 4).reshape(SLOTS, 4)
        starts = np.concatenate([[0], np.cumsum(counts.reshape(-1))[:-1]])
        rank = np.arange(len(key_s)) - starts[key_s]
        percore.append((key_s, idx_s, rank, counts))

    # per-(tile, class) shared D (max over cores)
    tileD = np.zeros((TILES, 4), np.int64)
    for t in range(TILES):
        lo, hi = t * P, (t + 1) * P
        for c in range(4):
            tileD[t, c] = max(
                max(int(percore[k][3][lo:hi, c].max()) for k in range(NCORES)), 1
            )

    # --- batches: pack tiles while tb * sum_c(max D_c) <= budget cols ---
    budget = t_batch  # interpreted as gather-column budget when > 32
    if budget <= 32:
        budget = 224
    batches = []
    t0 = 0
    while t0 < TILES:
        t1 = t0 + 1
        while t1 < TILES and t1 - t0 < 8:
            Dk = tileD[t0:t1 + 1].max(axis=0)
            if (t1 + 1 - t0) * int(Dk.sum()) > budget:
                break
            t1 += 1
        batches.append((t0, t1))
        t0 = t1

    meta = []
    col_off = 0
    for (t0, t1) in batches:
        tb = t1 - t0
        info = {"t0": t0, "t1": t1, "D": [], "off": [], "n": []}
        for c in range(4):
            D = int(tileD[t0:t1, c].max())
            n = tb * D * P
            info["D"].append(D)
            info["off"].append(col_off)
            info["n"].append(n)
            col_off += n // 16
        meta.append(info)
    W_total = col_off

    # --- pass 2: fill tables ---
    cores = []
    for k in range(NCORES):
        key_s, idx_s, rank, counts = percore[k]
        blocks = []
        for info in meta:
            t0, t1 = info["t0"], info["t1"]
            tb = t1 - t0
            lo, hi = t0 * P, t1 * P
            for c in range(4):
                D = info["D"][c]
                table = np.full((tb * D, P), zero_idx(c), np.int64)
                sel = (key_s % 4 == c) & (key_s // 4 >= lo) & (key_s // 4 < hi)
                sl = key_s[sel] // 4
                rr = rank[sel]
                tt = sl // P - t0
                pp = sl % P
                table[tt * D + rr, pp] = idx_s[sel]
                arr16 = table.reshape(-1).reshape(-1, 16).T
                assert table.max() <= 32767
                blocks.append(arr16.astype(np.int16))
        idx_all = np.ascontiguousarray(
            np.tile(np.concatenate(blocks, axis=1), (8, 1))
        )  # [128, W]: 16-partition block replicated for the 8 gpsimd cores
        assert idx_all.shape == (128, W_total)

        slot_node = np.full(SLOTS, -1, np.int64)
        mine = node_core == k
        slot_node[node_slot[mine]] = np.nonzero(mine)[0]
        dis_slot = np.zeros(SLOTS, np.float32)
        real = slot_node >= 0
        dis_slot[real] = dis[slot_node[real]]
        dis_pt = np.ascontiguousarray(dis_slot.reshape(TILES, P).T)  # [128, TILES]
        cores.append(
            dict(
                idx_all=idx_all,
                slot_node=slot_node,
                dis_pt=dis_pt,
                dis2_pt=dis_pt * dis_pt,
            )
        )

    return dict(
        cores=cores,
        meta=meta,
        node_core=node_core,
        node_slot=node_slot,
        dis=dis,
        W_total=W_total,
        t_batch=t_batch,
    )



"""Bass/Tile program for the sharded MixHop kernel (one SPMD program, 8 cores)."""

import concourse.bacc as bacc
import concourse.bass as bass
import concourse.tile as tile
from concourse import mybir
from concourse.masks import make_identity


F32 = mybir.dt.float32
I16 = mybir.dt.int16
AX = mybir.AxisListType
OP = mybir.AluOpType
ACT = mybir.ActivationFunctionType



def _ln(nc, sb, h, F, g_rep, be_rep, tag):
    """LayerNorm over free dim F of h [128, F] (in place).  Returns h."""
    mu = sb.tile([P, 1], F32, tag=f"{tag}_mu")
    nc.vector.tensor_reduce(mu[:], h[:], axis=AX.X, op=OP.add)
    nc.scalar.mul(mu[:], mu[:], 1.0 / F)
    cent = sb.tile([P, F], F32, tag=f"{tag}_cent")
    nc.vector.tensor_scalar(cent[:], h[:], mu[:], None, op0=OP.subtract)
    sq = sb.tile([P, F], F32, tag=f"{tag}_sq")
    ssum = sb.tile([P, 1], F32, tag=f"{tag}_ss")
    nc.scalar.activation(sq[:], cent[:], ACT.Square, accum_out=ssum[:])
    std = sb.tile([P, 1], F32, tag=f"{tag}_std")
    nc.scalar.activation(std[:], ssum[:], ACT.Sqrt, bias=prep.LN_EPS, scale=1.0 / F)
    rstd = sb.tile([P, 1], F32, tag=f"{tag}_rstd")
    nc.vector.reciprocal(rstd[:], std[:])
    nc.vector.tensor_scalar(cent[:], cent[:], rstd[:], None, op0=OP.mult)
    nc.vector.tensor_tensor(cent[:], cent[:], g_rep, op=OP.mult)
    nc.vector.tensor_tensor(cent[:], cent[:], be_rep, op=OP.add)
    return cent


def build(meta, W_total, n_hw_cores=8, max_hops=None, do_tail=True, do_gather=True, gather_classes=4, do_reduce=True):
    SLOTS, SHARD, TOT, TILES = prep.SLOTS, prep.SHARD, prep.TOT, prep.TILES
    IN_F, OUT_F, POWERS, MAXP = prep.IN_F, prep.OUT_F, prep.POWERS, prep.MAXP
    NC3 = len(POWERS) * HID  # 192

    nc = bacc.Bacc(
        "TRN2",
        target_bir_lowering=False,
        debug=False,
        num_devices=n_hw_cores,
    )

    x_in = nc.dram_tensor("x_core", [SLOTS, IN_F], F32, kind="ExternalInput")
    idx_in = nc.dram_tensor("idx_all", [128, W_total], I16, kind="ExternalInput")
    dis_in = nc.dram_tensor("dis_pt", [P, TILES], F32, kind="ExternalInput")
    dis2_in = nc.dram_tensor("dis2_pt", [P, TILES], F32, kind="ExternalInput")
    w1_in = nc.dram_tensor("W1", [IN_F, HID], F32, kind="ExternalInput")
    wc_in = nc.dram_tensor("Wcp", [HID, NC3], F32, kind="ExternalInput")
    w2_in = nc.dram_tensor("W2", [NC3, OUT_F], F32, kind="ExternalInput")
    # row constants, each replicated to 128 partitions:
    # b1[64] g1[64] be1[64] bc[192] g2[192] be2[192] b2[40]
    CV = HID * 3 + NC3 * 3 + OUT_F
    cvec_in = nc.dram_tensor("cvec", [P, CV], F32, kind="ExternalInput")
    out_t = nc.dram_tensor("out", [SLOTS, OUT_F], F32, kind="ExternalOutput")

    # internal DRAM state
    u_shard = [
        nc.dram_tensor(f"u_shard{i}", [SHARD, HID], F32, kind="Internal")
        for i in range(2)
    ]
    u_full = [
        nc.dram_tensor(f"u_full{i}", [TOT, HID], F32, kind="Internal")
        for i in range(2)
    ]
    s_save = {
        j: nc.dram_tensor(f"s_save{j}", [SLOTS, HID], F32, kind="Internal")
        for j in POWERS
    }

    rg = [list(range(n_hw_cores))]

    with tile.TileContext(nc) as tc:
        with tc.tile_pool(name="consts", bufs=1) as consts, \
             tc.tile_pool(name="sb", bufs=2) as sb, \
             tc.tile_pool(name="gat", bufs=2) as gat, \
             tc.tile_pool(name="ps", bufs=2, space="PSUM") as ps:
            # ---------------- constants ----------------
            zero_s = consts.tile([P, 1], F32)
            nc.vector.memset(zero_s[:], 0.0)
            nc.const_aps.aps[(F32, 0.0)] = zero_s[:]
            eps_s = consts.tile([P, 1], F32)
            nc.vector.memset(eps_s[:], prep.LN_EPS)
            nc.const_aps.aps[(F32, prep.LN_EPS)] = eps_s[:]
            ident = consts.tile([P, P], F32)
            make_identity(nc, ident[:])
            w1_sb = consts.tile([IN_F, HID], F32)
            nc.sync.dma_start(w1_sb[:], w1_in[:])
            wc_sb = consts.tile([HID, NC3], F32)
            nc.sync.dma_start(wc_sb[:], wc_in[:])
            w2a_sb = consts.tile([P, OUT_F], F32)
            nc.sync.dma_start(w2a_sb[:], w2_in[0:P, :])
            w2b_sb = consts.tile([NC3 - P, OUT_F], F32)
            nc.sync.dma_start(w2b_sb[:], w2_in[P:NC3, :])
            cvec = consts.tile([P, CV], F32)
            nc.sync.dma_start(cvec[:], cvec_in[:])
            o = [0, HID, 2 * HID, 3 * HID, 3 * HID + NC3, 3 * HID + 2 * NC3,
                 3 * HID + 3 * NC3, CV]
            b1_rep = cvec[:, o[0]:o[1]]
            g1_rep = cvec[:, o[1]:o[2]]
            be1_rep = cvec[:, o[2]:o[3]]
            bc_rep = cvec[:, o[3]:o[4]]
            g2_rep = cvec[:, o[4]:o[5]]
            be2_rep = cvec[:, o[5]:o[6]]
            b2_rep = cvec[:, o[6]:o[7]]
            idx_res = consts.tile([P, W_total], I16)
            nc.sync.dma_start(idx_res[:], idx_in[:])
            dis_sb = consts.tile([P, TILES], F32)
            nc.sync.dma_start(dis_sb[:], dis_in[:])
            dis2_sb = consts.tile([P, TILES], F32)
            nc.sync.dma_start(dis2_sb[:], dis2_in[:])
            zrow = consts.tile([1, HID], F32)
            nc.vector.memset(zrow[:], 0.0)
            for i in range(2):
                nc.sync.dma_start(u_shard[i][SLOTS:SHARD, :], zrow[:])
            # whole-shard state kept resident in SBUF across hops (self rows)
            t_sb = consts.tile([P, TILES * HID], F32)

            # ---------------- head: t0 = dis * LN(gelu(x@W1+b1)) ----------------
            for tt in range(TILES):
                xt = sb.tile([P, IN_F], F32, tag="xt")
                nc.sync.dma_start(xt[:], x_in[tt * P:(tt + 1) * P, :])
                tp_ps = ps.tile([P, P], F32, tag="tp")
                nc.tensor.transpose(tp_ps[:], xt[:], ident[:])
                xT = sb.tile([P, P], F32, tag="xT")
                nc.vector.tensor_copy(xT[:], tp_ps[:])
                h_ps = ps.tile([P, HID], F32, tag="mm")
                nc.tensor.matmul(h_ps[:], lhsT=xT[:], rhs=w1_sb[:], start=True, stop=True)
                h = sb.tile([P, HID], F32, tag="h")
                nc.vector.tensor_tensor(h[:], h_ps[:], b1_rep, op=OP.add)
                nc.scalar.activation(h[:], h[:], ACT.Gelu)
                hn = _ln(nc, sb, h, HID, g1_rep, be1_rep, "ln1")
                t0s = t_sb[:, tt * HID:(tt + 1) * HID]
                nc.vector.tensor_scalar(t0s, hn[:], dis_sb[:, tt:tt + 1], None, op0=OP.mult)
                nc.sync.dma_start(u_shard[0][tt * P:(tt + 1) * P, :], t0s)

            # ---------------- hops ----------------
            for hop in range(1, (MAXP if max_hops is None else max_hops) + 1):
                u_in = u_shard[(hop - 1) % 2]
                u_out = u_shard[hop % 2]
                uf = u_full[(hop - 1) % 2]
                if n_hw_cores > 1:
                    nc.gpsimd.collective_compute(
                        "AllGather",
                        OP.bypass,
                        replica_groups=rg,
                        ins=[u_in[:].opt()],
                        outs=[uf[:].opt()],
                    )
                else:
                    nc.sync.dma_start(uf[0:SHARD, :], u_in[:])
                uf4 = uf[:].rearrange("(a b) f -> a (b f)", b=4)
                for info in meta:
                    if not do_gather:
                        break
                    t0, t1 = info["t0"], info["t1"]
                    tb = t1 - t0
                    Gs = []
                    for c in range(gather_classes):
                        D, off, n = info["D"][c], info["off"][c], info["n"][c]
                        G = gat.tile([P, tb * D * HID], F32, tag=f"g{c}")
                        nc.gpsimd.dma_gather(
                            G[:].rearrange("p (c f) -> p c f", f=HID),
                            uf4[:, c * HID:(c + 1) * HID],
                            idx_res[:, off:off + n // 16],
                            n,
                            n,
                            HID,
                            elem_step=4 * HID,
                            single_packet=False,
                        )
                        Gs.append((G[:], D))
                    self_sl = t_sb[:, t0 * HID:t1 * HID]
                    acc = sb.tile([P, tb * HID], F32, tag="acc")
                    tmp = sb.tile([P, tb * HID], F32, tag="rtmp")
                    if not do_reduce:
                        nc.vector.tensor_copy(acc[:], self_sl)
                    for c in range(gather_classes if do_reduce else 0):
                        gslice, D = Gs[c]
                        gv = gslice.rearrange("p (t d f) -> p t f d", t=tb, d=D, f=HID)
                        dst = acc if c == 0 else tmp
                        nc.vector.tensor_reduce(dst[:], gv, axis=AX.X, op=OP.add)
                        if c > 0:
                            nc.vector.tensor_tensor(acc[:], acc[:], tmp[:], op=OP.add)
                    nc.vector.tensor_tensor(acc[:], acc[:], self_sl, op=OP.add)
                    if hop in POWERS:
                        # tap: o_j = (dis * acc) @ Wc_j, computed here where the
                        # PE is idle (the hop is DMA-bound); tail just concats.
                        ji = POWERS.index(hop)
                        for t in range(tb):
                            st = sb.tile([P, HID], F32, tag="st")
                            nc.vector.tensor_scalar(
                                st[:], acc[:, t * HID:(t + 1) * HID],
                                dis_sb[:, t0 + t:t0 + t + 1], None, op0=OP.mult,
                            )
                            tpv = ps.tile([HID, P], F32, tag="tp")
                            nc.tensor.transpose(tpv[:], st[:], ident[:])
                            sT = sb.tile([HID, P], F32, tag="sTh")
                            nc.vector.tensor_copy(sT[:], tpv[:])
                            o_ps = ps.tile([P, HID], F32, tag="mm")
                            nc.tensor.matmul(
                                o_ps[:], lhsT=sT[:],
                                rhs=wc_sb[:, ji * HID:(ji + 1) * HID],
                                start=True, stop=True,
                            )
                            osb = sb.tile([P, HID], F32, tag="osb")
                            nc.vector.tensor_copy(osb[:], o_ps[:])
                            nc.sync.dma_start(
                                s_save[hop][(t0 + t) * P:(t0 + t + 1) * P, :],
                                osb[:],
                            )
                    if hop < MAXP:
                        for t in range(tb):
                            nc.vector.tensor_scalar(
                                t_sb[:, (t0 + t) * HID:(t0 + t + 1) * HID],
                                acc[:, t * HID:(t + 1) * HID],
                                dis2_sb[:, t0 + t:t0 + t + 1],
                                None,
                                op0=OP.mult,
                            )
                        nc.sync.dma_start(
                            u_out[t0 * P:t1 * P, :].rearrange("(t p) f -> p t f", p=P),
                            t_sb[:, t0 * HID:t1 * HID].rearrange("p (t f) -> p t f", f=HID),
                        )

            # ---------------- tail ----------------
            for tt in range(TILES if do_tail else 0):
                scat = sb.tile([P, NC3], F32, tag="scat")
                for ji, j in enumerate(POWERS):
                    nc.sync.dma_start(
                        scat[:, ji * HID:(ji + 1) * HID],
                        s_save[j][tt * P:(tt + 1) * P, :],
                    )
                hc = sb.tile([P, NC3], F32, tag="hc")
                nc.vector.tensor_tensor(hc[:], scat[:], bc_rep, op=OP.add)
                nc.scalar.activation(hc[:], hc[:], ACT.Gelu)
                hn = _ln(nc, sb, hc, NC3, g2_rep, be2_rep, "ln2")
                tpc = ps.tile([P, P], F32, tag="tp")
                nc.tensor.transpose(tpc[:], hn[:, 0:P], ident[:])
                hTa = sb.tile([P, P], F32, tag="hTa")
                nc.vector.tensor_copy(hTa[:], tpc[:])
                tpd = ps.tile([NC3 - P, P], F32, tag="tp")
                nc.tensor.transpose(tpd[:], hn[:, P:NC3], ident[:])
                hTb = sb.tile([NC3 - P, P], F32, tag="hTb")
                nc.vector.tensor_copy(hTb[:], tpd[:])
                o_ps = ps.tile([P, OUT_F], F32, tag="mm")
                nc.tensor.matmul(o_ps[:], lhsT=hTa[:], rhs=w2a_sb[:], start=True, stop=False)
                nc.tensor.matmul(o_ps[:], lhsT=hTb[:], rhs=w2b_sb[:], start=False, stop=True)
                ot = sb.tile([P, OUT_F], F32, tag="ot")
                nc.vector.tensor_tensor(ot[:], o_ps[:], b2_rep, op=OP.add)
                nc.sync.dma_start(out_t[tt * P:(tt + 1) * P, :], ot[:])

    nc.compile()
    return nc


def make_in_maps(inputs, pp):
    """Build the 8 per-core input dicts."""
    POWERS = prep.POWERS
    x = np.asarray(inputs["x"], np.float32)
    Wc = np.asarray(inputs["Wc"], np.float32)
    bc = np.asarray(inputs["bc"], np.float32)
    wcp = np.concatenate([Wc[j] for j in POWERS], axis=1)  # [64, 192]
    bccat = np.concatenate([bc[j] for j in POWERS], axis=0)  # [192]
    cv = np.concatenate([
        np.asarray(inputs["b1"], np.float32),
        np.asarray(inputs["g1"], np.float32),
        np.asarray(inputs["be1"], np.float32),
        bccat,
        np.asarray(inputs["g2"], np.float32),
        np.asarray(inputs["be2"], np.float32),
        np.asarray(inputs["b2"], np.float32),
    ])
    cvec = np.tile(cv[None, :], (P, 1)).copy()

    in_maps = []
    for k in range(prep.NCORES):
        ck = pp["cores"][k]
        sn = ck["slot_node"]
        xk = np.zeros((prep.SLOTS, prep.IN_F), np.float32)
        xk[sn >= 0] = x[sn[sn >= 0]]
        in_maps.append(
            dict(
                x_core=xk,
                idx_all=ck["idx_all"],
                dis_pt=ck["dis_pt"],
                dis2_pt=ck["dis2_pt"],
                W1=np.asarray(inputs["W1"], np.float32),
                Wcp=np.ascontiguousarray(wcp),
                W2=np.asarray(inputs["W2"], np.float32),
                cvec=cvec,
            )
        )
    return in_maps


def assemble_output(results, pp):
    """results: list of per-core dicts with 'out' -> full [N, OUT_F]."""
    out = np.zeros((prep.N, prep.OUT_F), np.float32)
    for k in range(prep.NCORES):
        sn = pp["cores"][k]["slot_node"]
        o = results[k]["out"]
        out[sn[sn >= 0]] = o[sn >= 0]
    return out


_BUILD_CACHE = {}


def kernel(**inputs):
    pp = preprocess(inputs["edge_index"], t_batch=144)
    key = (pp["W_total"], tuple(tuple(i["D"]) for i in pp["meta"]))
    nc = _BUILD_CACHE.get(key)
    if nc is None:
        nc = build(pp["meta"], pp["W_total"])
        _BUILD_CACHE[key] = nc
    in_maps = make_in_maps(inputs, pp)
    from concourse import bass_utils

    res = bass_utils.run_bass_kernel_spmd(nc, in_maps, core_ids=list(range(8)))
    return assemble_output(res.results, pp)
